# revision 41
# baseline (speedup 1.0000x reference)
"""Bass/Trainium2 kernel for nn_LocalAggregator (GNN message passing).

Math per batch b (hidden [64,128], adj [64,64] in {0..4}, a [4,128]):
    e_k[i,j] = leakyrelu_{0.2}( sum_d hidden[i,d]*hidden[j,d]*a[k,d] )
    alpha    = softmax_j( where(adj==k+1, e_k, -9e15) )
    out      = alpha @ hidden

Device strategy (8 cores, pure batch data-parallel, 64 batches/core).
Batches are fused in PAIRS (2 batches = 128 nodes -> full-width 128x128
matmuls, half the matmul instructions; cross-batch terms are garbage
that the mask kills), and processed in OCTs (4 pairs = 8 batches) so
element-wise ops run on [128, 2048] tiles that amortize per-op
overheads.

Per oct (tiles: hT bf16 [d,(pair,i)], hh fp16 [j2b,(pair,d+ones)],
A [j2b,(pair,k,i)] additive mask, shipped int8 {0,-40} and cast to
fp16 by the SWDGE during the DMA on the otherwise-idle gpsimd queue):
  - w_all[d,(pair,k,i)] = hT * a_k          (4 tensor_scalar ops on DVE)
  - e2[j2b,(k,i)] = hT_pair^T @ w_all_pair  (1 matmul per pair, f32 PSUM;
    e_k symmetric -> tile read as [j,(k,i)] is e_k[i,j])
  - lr = Prelu(e2) evacuates PSUM on ACT (fp16)
  - esel = max_k (lr + A): the selected lr where adj==k+1 (exact),
    else <= -34; exp(esel) underflows to exactly 0 in fp16 for masked
    and cross-batch entries. (leakyrelu commutes with selection.)
  - out_pair[i,(d,s)] = w_pair^T @ [hh|1]: unnormalized rows + softmax
    denominator s_i, shipped fp16; the HOST divides.
PSUM note: every matmul output region must stay inside one 2 KiB PSUM
bank (the 256-col f32 slots in osum are load-bearing).
"""

import numpy as np
import ml_dtypes

import concourse.bass as bass
import concourse.tile as tile
from concourse import bacc, mybir
from concourse._compat import with_exitstack
from concourse.bass_utils import run_bass_kernel_spmd

F16 = mybir.dt.float16
BF16 = mybir.dt.bfloat16
F32 = mybir.dt.float32
ALU = mybir.AluOpType
ACTF = mybir.ActivationFunctionType

B, N, D, K = 512, 64, 128, 4
NCORES = 8
BPC = B // NCORES          # 64 batches per core
NOCT = BPC // 8            # 8 octs of 8 batches (4 pairs) per core
HHW = 132                  # hidden cols + ones col + pad
CIN = 512 + 4 * HHW + 2048  # blob cols: hT(512) | hh(528) | A(2048)
MASKV = -40.0


@with_exitstack
def _kernel_body(ctx, tc, blob_d, hT_d, am_d, aT_d, out_d):
    nc = tc.nc

    const_pool = ctx.enter_context(tc.tile_pool(name="const", bufs=1))
    in_pool = ctx.enter_context(tc.tile_pool(name="inp", bufs=8))
    mask_pool = ctx.enter_context(tc.tile_pool(name="mask", bufs=4))
    work_pool = ctx.enter_context(tc.tile_pool(name="work", bufs=4))
    psum_pool = ctx.enter_context(tc.tile_pool(name="psum", bufs=2, space="PSUM"))
    opsum_pool = ctx.enter_context(tc.tile_pool(name="opsum", bufs=2, space="PSUM"))
    out_pool = ctx.enter_context(tc.tile_pool(name="outp", bufs=3))

    a_sb = const_pool.tile([128, 4], F32)          # a^T : [d, k]
    nc.sync.dma_start(out=a_sb[:], in_=aT_d[:, :])

    for q in range(NOCT):
        # split load: compute-critical hT+hh on the sync HWDGE queue; the
        # big mask ships int8 and is cast to fp16 by the SWDGE during DMA
        # (half the HBM bytes, and on the otherwise-idle gpsimd queue)
        amask_t = mask_pool.tile([128, 2048], F16, tag="amask")
        nc.gpsimd.dma_start(out=amask_t[:], in_=am_d[q])
        hT_t = in_pool.tile([128, 512], BF16, tag="hT")
        nc.sync.dma_start(out=hT_t[:], in_=hT_d[q])
        blob = in_pool.tile([128, 4 * HHW], F16, tag="blob")
        nc.sync.dma_start(out=blob[:], in_=blob_d[q])
        hT = hT_t[:]                              # [d, (pair, i)] bf16
        hh = blob[:, 0 : 4 * HHW]                 # [j2b, (pair, d+ones)]
        amask = amask_t[:]                        # [j2b, (pair, k, i)]

        # ---- w_all[d, (pair, k, i)] = hT * a_k (pair-major storage:
        #      strided builds, but each e-matmul rhs is contiguous) ----
        w_all = work_pool.tile([128, 2048], BF16, tag="w_all")
        wv = w_all[:].rearrange("p (a k i) -> p a k i", a=4, k=4)
        hTv = hT.rearrange("p (a i) -> p a i", a=4)
        for k in range(K):
            nc.vector.tensor_scalar(
                wv[:, :, k, :], hTv, a_sb[:, k : k + 1], None, ALU.mult)

        # ---- e2 per pair (PSUM f32), Prelu-evacuated to lr (fp16) ----
        lr = work_pool.tile([128, 2048], F16, tag="lr")
        for duo in range(2):
            e2 = psum_pool.tile([128, 1024], F32, tag="e2")
            for pp in range(2):
                p = 2 * duo + pp
                nc.tensor.matmul(
                    e2[:, pp * 512 : (pp + 1) * 512],
                    lhsT=hT[:, p * 128 : (p + 1) * 128],
                    rhs=w_all[:, p * 512 : (p + 1) * 512],
                    start=True, stop=True,
                )
            nc.scalar.activation(
                lr[:, duo * 1024 : (duo + 1) * 1024], e2[:],
                ACTF.Prelu, alpha=0.2)

        # ---- esel[j, (pair, i)] = max_k (lr + A) ----
        # lr columns are (pair, k, i); A's are (pair, k, i) as well.
        # The final oct runs the chain per duo (halved tiles) so the
        # pipeline drain tail is shorter; earlier octs use full-width ops.
        z = work_pool.tile([128, 2048], F16, tag="z")
        t2 = work_pool.tile([128, 1024], F16, tag="t2")
        esel = work_pool.tile([128, 512], F16, tag="esel")
        w = work_pool.tile([128, 512], F16, tag="w")
        nduo = 2 if q == NOCT - 1 else 1
        for s in range(nduo):
            na = 4 // nduo                        # pairs per chain step
            zs = z[:, s * na * 512 : (s + 1) * na * 512]
            nc.vector.tensor_tensor(
                zs, lr[:, s * na * 512 : (s + 1) * na * 512],
                amask[:, s * na * 512 : (s + 1) * na * 512], ALU.add)
            zv = zs.rearrange("p (a k i) -> p a k i", a=na, k=4)
            t2v = (t2[:, s * na * 256 : (s + 1) * na * 256]
                   .rearrange("p (a k i) -> p a k i", a=na, k=2))
            nc.vector.tensor_tensor(t2v, zv[:, :, 0:2, :], zv[:, :, 2:4, :],
                                    ALU.max)
            ev = (esel[:, s * na * 128 : (s + 1) * na * 128]
                  .rearrange("p (a i) -> p a i", a=na))
            nc.vector.tensor_tensor(ev, t2v[:, :, 0, :], t2v[:, :, 1, :],
                                    ALU.max)
            # w = exp(esel): masked entries -> exactly 0 in fp16
            nc.scalar.activation(
                w[:, s * na * 128 : (s + 1) * na * 128],
                esel[:, s * na * 128 : (s + 1) * na * 128], ACTF.Exp)

        # ---- out_pair[i, (d,s)] = sum_j w[j,i] hh[j,c]; col 128 = s_i ----
        osb = out_pool.tile([128, 4 * HHW], F16, tag="osb")
        osum = opsum_pool.tile([128, 1024], F32, tag="osum")
        for p in range(4):
            nc.tensor.matmul(
                osum[:, p * 256 : p * 256 + HHW],
                lhsT=w[:, p * 128 : (p + 1) * 128],
                rhs=hh[:, p * HHW : (p + 1) * HHW],
                start=True, stop=True,
            )
        osbv = osb[:].rearrange("p (a c) -> p a c", a=4)
        osumv = osum[:].rearrange("p (a c) -> p a c", a=4)[:, :, 0:HHW]
        if q % 2 == 0:
            nc.scalar.activation(osbv, osumv, ACTF.Copy)
        else:
            nc.vector.tensor_scalar(osbv, osumv, 1.0, None, ALU.mult)
        nc.sync.dma_start(out=out_d[q], in_=osb[:])


def build_nc():
    nc = bacc.Bacc("TRN2", target_bir_lowering=False, debug=False)
    blob_d = nc.dram_tensor("blob", [NOCT, 128, 4 * HHW], F16,
                            kind="ExternalInput").ap()
    hT_d = nc.dram_tensor("hT", [NOCT, 128, 512], BF16,
                          kind="ExternalInput").ap()
    am_d = nc.dram_tensor("am", [NOCT, 128, 2048], mybir.dt.int8,
                          kind="ExternalInput").ap()
    aT_d = nc.dram_tensor("at", [128, 4], F32, kind="ExternalInput").ap()
    out_d = nc.dram_tensor("out", [NOCT, 128, 4 * HHW], F16,
                           kind="ExternalOutput").ap()
    with tile.TileContext(nc) as tc:
        _kernel_body(tc, blob_d, hT_d, am_d, aT_d, out_d)
    nc.compile()
    return nc


def prep_inputs(hidden, adj, a):
    """Host-side packing: fp16 casts, pair-fused block layouts, masks."""
    hidden = np.asarray(hidden, dtype=np.float32)
    adj = np.asarray(adj)
    a = np.asarray(a, dtype=np.float32)

    h16 = hidden.astype(np.float16)                          # [B, 64, 128]

    # hT[pg, d, v] with v = u*64+i, batch = 2*pg + u
    hT = (h16.transpose(0, 2, 1)                             # [b, d, i]
          .reshape(B // 2, 2, D, N)                          # [pg, u, d, i]
          .transpose(0, 2, 1, 3)                             # [pg, d, u, i]
          .reshape(B // 2, D, 2 * N))

    # hh[pg, v, c]: row v = h[2pg + v//64, v%64, :] + ones col
    hh = np.zeros((B // 2, 2 * N, HHW), dtype=np.float16)
    hh[:, :, 0:D] = h16.reshape(B // 2, 2 * N, D)
    hh[:, :, D] = np.float16(1.0)

    # A[pg, x, k*128+y] = 0 where block-diag transposed adj == k+1 else -40
    # (x = j2b, y = i2b; cross-batch blocks are all -40)
    at = adj.transpose(0, 2, 1)                              # at[b, j, i]
    A = np.full((B // 2, 2 * N, K, 2 * N), MASKV, dtype=np.int8)
    ks = np.arange(1, K + 1)[None, None, None, :]            # [1,1,1,k]
    ohA = (at[0::2][:, :, :, None] == ks)                    # [pg, j, i, k]
    ohB = (at[1::2][:, :, :, None] == ks)
    ohAt = np.transpose(ohA, (0, 1, 3, 2))                   # [pg, j, k, i]
    ohBt = np.transpose(ohB, (0, 1, 3, 2))
    A[:, 0:N, :, 0:N][ohAt] = 0
    A[:, N:2 * N, :, N:2 * N][ohBt] = 0
    A = A.reshape(B // 2, 2 * N, K * 2 * N)

    aT = np.ascontiguousarray(a.T).astype(np.float32)        # [128, 4]

    # blob[oct, 128, CIN] per core: hT(4 pairs) | hh | A
    PPC = BPC // 2                                           # 32 pairs per core
    in_maps = []
    for c in range(NCORES):
        psl = slice(c * PPC, (c + 1) * PPC)
        hT_c = hT[psl].reshape(NOCT, 4, D, 2 * N)
        hh_c = hh[psl].reshape(NOCT, 4, 2 * N, HHW)
        A_c = A[psl].reshape(NOCT, 4, 2 * N, K * 2 * N)
        blob = np.ascontiguousarray(
            hh_c.transpose(0, 2, 1, 3).reshape(NOCT, 128, 4 * HHW))
        hTb = np.ascontiguousarray(
            hT_c.transpose(0, 2, 1, 3).reshape(NOCT, 128, 512)
            .astype(np.float32).astype(ml_dtypes.bfloat16))
        am = np.ascontiguousarray(
            A_c.transpose(0, 2, 1, 3).reshape(NOCT, 128, 2048))
        in_maps.append({"blob": blob, "hT": hTb, "am": am, "at": aT})
    return in_maps


_NC_CACHE = {}


def run_device(hidden, adj, a, **spmd_kwargs):
    if "nc" not in _NC_CACHE:
        _NC_CACHE["nc"] = build_nc()
    nc = _NC_CACHE["nc"]
    in_maps = prep_inputs(hidden, adj, a)
    res = run_bass_kernel_spmd(nc, in_maps, list(range(NCORES)), **spmd_kwargs)
    outs = []
    for c in range(NCORES):
        o = res.results[c]["out"].astype(np.float32)         # [NOCT, 128, 528]
        o = (o.reshape(NOCT, 2, N, 4, HHW)                   # [q, u, i, pair, c]
             .transpose(0, 3, 1, 2, 4)                       # [q, pair, u, i, c]
             .reshape(BPC, N, HHW))
        outs.append(o[:, :, 0:D] / o[:, :, D:D + 1])
    out = np.concatenate(outs, axis=0)
    return out.reshape(B, N, D).astype(np.float32), res


def kernel(hidden, adj, a):
    out, _ = run_device(hidden, adj, a)
    return out
